# revision 1
# baseline (speedup 1.0000x reference)
"""Trainium2 Bass kernel for nn_LlamaQuantizedMLP (int4 fake-quant SwiGLU MLP).

Strategy
--------
The reference computes per-row int4 fake quantization of each weight
(scale = max|w|/7, q = clip(round(w/scale), -8, 7), w' = q*scale) followed by
  gate = x @ wg'.T ; up = x @ wu'.T ; h = silu(gate)*up ; y = h @ wd'.T

Host side (inside kernel()): compute scales + integer q values (numpy f32,
bit-matching the jax f32 reference ops), and ship the q values to the device
as *bf16* — integers in [-8, 7] are exactly representable in bf16, so this
halves HBM traffic with zero quantization error.  The per-row scales are
folded in after the matmuls (gate/up: on device before silu; down: on host
after the cross-core reduction).

x (and the SwiGLU intermediate h) are split into bf16 (hi, lo) pairs with
hi + lo == value to ~2^-18 relative error; both halves accumulate into the
same PSUM region, so matmul numerics match f32 to ~1e-6.

Sharding: tensor parallel over the intermediate dim (11008 = 8 cores x 1376).
Each core holds column shards of gate/up and a row shard of down and emits a
partial [8, 4096] output; the host sums the 8 partials (cheaper than a
device AllReduce for 128 KB) and applies the down scales.

Weights are pre-transposed and pre-tiled on the host into [128, KO, N]
partition-major layouts so every DMA descriptor is a long contiguous run.
"""

import numpy as np
import ml_dtypes

import concourse.bacc as bacc
import concourse.mybir as mybir
from concourse.tile import TileContext
from concourse import bass_utils

BF16 = mybir.dt.bfloat16
F32 = mybir.dt.float32
NP_BF16 = ml_dtypes.bfloat16

NCORES = 8
NTILE = 512  # PSUM bank / matmul moving-operand chunk


class Cfg:
    def __init__(self, b=8, h=4096, i_full=11008, wblk=8, dblk=2):
        assert h % 128 == 0 and i_full % NCORES == 0
        self.B = b
        self.H = h
        self.I_FULL = i_full
        self.I_SH = i_full // NCORES
        self.KC = h // 128                      # gate/up contraction chunks
        self.IC = (self.I_SH + 127) // 128      # down contraction chunks
        self.I_LAST = self.I_SH - 128 * (self.IC - 1)
        self.WBLK = min(wblk, self.KC)          # gate/up ko per DMA
        self.DBLK = min(dblk, self.IC)          # down ko per DMA
        assert self.KC % self.WBLK == 0
        # gate accumulates in PSUM cols [0, I_SH); up at bank-aligned UP0
        self.UP0 = ((self.I_SH + NTILE - 1) // NTILE) * NTILE
        assert self.UP0 + self.I_SH <= 2 * 4096  # 8 banks of 512 f32 (b>=... )
        assert self.UP0 + self.I_SH <= 4096 or h <= 4096
        self.ACC_W = max(self.UP0 + ((self.I_SH + NTILE - 1) // NTILE) * NTILE,
                         self.H)


FULL = Cfg()


def _nslices(n):
    return [(s, min(s + NTILE, n)) for s in range(0, n, NTILE)]


def build(nc, cfg):
    """Emit the per-core SPMD program (identical on all cores; data differs)."""
    B, H, I_SH, KC, IC = cfg.B, cfg.H, cfg.I_SH, cfg.KC, cfg.IC

    xt_hi = nc.dram_tensor("xt_hi", [128, KC, B], BF16, kind="ExternalInput")
    gcorr = nc.dram_tensor("gcorr", [B, I_SH], F32, kind="ExternalInput")
    ucorr = nc.dram_tensor("ucorr", [B, I_SH], F32, kind="ExternalInput")
    qgt = nc.dram_tensor("qgt", [128, KC, I_SH], BF16, kind="ExternalInput")
    qut = nc.dram_tensor("qut", [128, KC, I_SH], BF16, kind="ExternalInput")
    qdt = nc.dram_tensor("qdt", [128, IC, H], BF16, kind="ExternalInput")
    sg = nc.dram_tensor("sg", [B, I_SH], F32, kind="ExternalInput")
    su = nc.dram_tensor("su", [B, I_SH], F32, kind="ExternalInput")
    y = nc.dram_tensor("y", [B, H], F32, kind="ExternalOutput")

    with TileContext(nc) as tc:
        with (
            tc.tile_pool(name="xs", bufs=1) as xs_pool,
            tc.tile_pool(name="w", bufs=2) as w_pool,
            tc.tile_pool(name="act", bufs=1) as act_pool,
            tc.tile_pool(name="ps", bufs=1, space="PSUM") as ps_pool,
        ):
            xhi = xs_pool.tile([128, KC, B], BF16, tag="xhi")
            nc.sync.dma_start(out=xhi[:], in_=xt_hi[:])
            gc_t = xs_pool.tile([B, I_SH], F32, tag="gc")
            nc.sync.dma_start(out=gc_t[:], in_=gcorr[:])
            uc_t = xs_pool.tile([B, I_SH], F32, tag="uc")
            nc.sync.dma_start(out=uc_t[:], in_=ucorr[:])
            sg_t = xs_pool.tile([B, I_SH], F32, tag="sg")
            nc.sync.dma_start(out=sg_t[:], in_=sg[:])
            su_t = xs_pool.tile([B, I_SH], F32, tag="su")
            nc.sync.dma_start(out=su_t[:], in_=su[:])

            # ---------------- phase 1: gate & up ----------------
            acc = ps_pool.tile([B, cfg.ACC_W], F32, tag="acc")
            for kb in range(KC // cfg.WBLK):
                wg_t = w_pool.tile([128, cfg.WBLK, I_SH], BF16, tag="wg")
                nc.sync.dma_start(
                    out=wg_t[:], in_=qgt[:, kb * cfg.WBLK:(kb + 1) * cfg.WBLK, :])
                wu_t = w_pool.tile([128, cfg.WBLK, I_SH], BF16, tag="wu")
                nc.sync.dma_start(
                    out=wu_t[:], in_=qut[:, kb * cfg.WBLK:(kb + 1) * cfg.WBLK, :])
                for j in range(cfg.WBLK):
                    ko = kb * cfg.WBLK + j
                    lhs = xhi[:, ko, :]
                    st = ko == 0
                    sp = ko == KC - 1
                    for n0, n1 in _nslices(I_SH):
                        nc.tensor.matmul(
                            acc[:, n0:n1], lhs, wg_t[:, j, n0:n1],
                            start=st, stop=sp)
                    for n0, n1 in _nslices(I_SH):
                        nc.tensor.matmul(
                            acc[:, cfg.UP0 + n0:cfg.UP0 + n1], lhs,
                            wu_t[:, j, n0:n1], start=st, stop=sp)

            # ---------------- SwiGLU ----------------
            gate_raw = act_pool.tile([B, I_SH], F32, tag="graw")
            nc.vector.tensor_add(out=gate_raw[:], in0=acc[0:B, 0:I_SH], in1=gc_t[:])
            gate_sc = act_pool.tile([B, I_SH], F32, tag="gsc")
            nc.vector.tensor_mul(out=gate_sc[:], in0=gate_raw[:], in1=sg_t[:])
            sigm = act_pool.tile([B, I_SH], F32, tag="sigm")
            nc.scalar.activation(
                out=sigm[:], in_=gate_sc[:],
                func=mybir.ActivationFunctionType.Sigmoid)
            silu = act_pool.tile([B, I_SH], F32, tag="silu")
            nc.vector.tensor_mul(out=silu[:], in0=gate_sc[:], in1=sigm[:])
            up_raw = act_pool.tile([B, I_SH], F32, tag="uraw")
            nc.vector.tensor_add(
                out=up_raw[:], in0=acc[0:B, cfg.UP0:cfg.UP0 + I_SH], in1=uc_t[:])
            up_sc = act_pool.tile([B, I_SH], F32, tag="usc")
            nc.vector.tensor_mul(out=up_sc[:], in0=up_raw[:], in1=su_t[:])
            h_pad = act_pool.tile([32, I_SH], F32, tag="hpad")
            nc.gpsimd.memset(h_pad[:], 0.0)
            nc.vector.tensor_mul(out=h_pad[0:B, :], in0=silu[:], in1=up_sc[:])

            # ------- transpose h to [i, b] via DVE 32x32 blocks -------
            hT_f = act_pool.tile([128, IC, 32], F32, tag="hTf")
            nc.gpsimd.memset(hT_f[:], 0.0)  # rows beyond I_LAST in last chunk
            for c in range(I_SH // 32):
                ko, qoff = (32 * c) // 128, (32 * c) % 128
                nc.vector.transpose(
                    out=hT_f[qoff:qoff + 32, ko, :],
                    in_=h_pad[:, 32 * c:32 * c + 32])
            hT_hi = act_pool.tile([128, IC, B], BF16, tag="hThi")
            nc.vector.tensor_copy(out=hT_hi[:], in_=hT_f[:, :, 0:B])
            hT_hi_f = act_pool.tile([128, IC, B], F32, tag="hThif")
            nc.vector.tensor_copy(out=hT_hi_f[:], in_=hT_hi[:])
            hT_lo = act_pool.tile([128, IC, B], BF16, tag="hTlo")
            nc.vector.tensor_sub(out=hT_lo[:], in0=hT_f[:, :, 0:B], in1=hT_hi_f[:])

            # ---------------- phase 2: down ----------------
            y_ps = ps_pool.tile([B, H], F32, tag="acc")
            n_dblk = (IC + cfg.DBLK - 1) // cfg.DBLK
            for db in range(n_dblk):
                nko = min(cfg.DBLK, IC - db * cfg.DBLK)
                wd_t = w_pool.tile([128, cfg.DBLK, H], BF16, tag="wd")
                nc.sync.dma_start(
                    out=wd_t[:, 0:nko, :],
                    in_=qdt[:, db * cfg.DBLK:db * cfg.DBLK + nko, :])
                for j in range(nko):
                    ko = db * cfg.DBLK + j
                    kk = 128 if ko < IC - 1 else cfg.I_LAST
                    for half, hT in ((0, hT_hi), (1, hT_lo)):
                        lhs = hT[0:kk, ko, :]
                        st = ko == 0 and half == 0
                        sp = ko == IC - 1 and half == 1
                        for n0, n1 in _nslices(H):
                            nc.tensor.matmul(
                                y_ps[:, n0:n1], lhs, wd_t[0:kk, j, n0:n1],
                                start=st, stop=sp)

            y_sb = act_pool.tile([B, H], F32, tag="ysb")
            nc.vector.tensor_copy(out=y_sb[:], in_=y_ps[:])
            nc.sync.dma_start(out=y[:], in_=y_sb[:])

    nc.compile()
    return nc


# ---------------------------------------------------------------------------
# host-side preparation
# ---------------------------------------------------------------------------

def _quant(w):
    """Reference int4 fake-quant, split into integer q (f32) and scale."""
    w = np.asarray(w, np.float32)
    scale = (np.max(np.abs(w), axis=1, keepdims=True) / np.float32(7.0)).astype(
        np.float32)
    scale = np.maximum(scale, np.float32(np.finfo(np.float32).tiny))
    q = np.clip(np.round((w / scale).astype(np.float32)), -8.0, 7.0).astype(
        np.float32)
    return q, scale


def _tile_gu(q_sh, cfg):
    """[I_SH, H] q values -> [128, KC, I_SH] bf16 (partition-major)."""
    t = np.ascontiguousarray(q_sh.T)                 # [H, I_SH]
    t = t.reshape(cfg.KC, 128, cfg.I_SH).transpose(1, 0, 2)
    return np.ascontiguousarray(t.astype(NP_BF16))


def _tile_d(qd_sh, cfg):
    """[H, I_SH] down q values -> [128, IC, H] bf16, zero-padded rows."""
    t = qd_sh.T                                      # [I_SH, H]
    pad = np.zeros((cfg.IC * 128, cfg.H), np.float32)
    pad[0:cfg.I_SH] = t
    pad = pad.reshape(cfg.IC, 128, cfg.H).transpose(1, 0, 2)
    return np.ascontiguousarray(pad.astype(NP_BF16))


def _pair_x(x, cfg):
    """x [B, H] -> bf16 hi tiles [128, KC, B] + f32 residual [B, H]."""
    xT = np.ascontiguousarray(x.reshape(cfg.B, cfg.H).T)  # [H, B]
    hi = xT.astype(NP_BF16)
    lo = xT - hi.astype(np.float32)                        # f32 residual
    hi_t = np.ascontiguousarray(
        hi.reshape(cfg.KC, 128, cfg.B).transpose(1, 0, 2))
    return hi_t, np.ascontiguousarray(lo.T)                # lo as [B, H]


def make_in_maps(x, w_gate, w_up, w_down, cfg):
    """Returns (in_maps for 8 cores, down-scale vector [H])."""
    qg, sgf = _quant(w_gate)
    qu, suf = _quant(w_up)
    qd, sdf = _quant(w_down)
    xt_hi, x_lo = _pair_x(np.asarray(x, np.float32), cfg)
    # exact f32 correction for the bf16 rounding of x: lo @ q.T (tiny rank-B
    # term, ~0.25% of the FLOPs) -- computed host-side so the device streams
    # each gate/up weight only once.
    gcorr_full = x_lo @ qg.T                               # [B, I_FULL]
    ucorr_full = x_lo @ qu.T
    in_maps = []
    for c in range(NCORES):
        isl = slice(c * cfg.I_SH, (c + 1) * cfg.I_SH)
        in_maps.append({
            "xt_hi": xt_hi,
            "gcorr": np.ascontiguousarray(gcorr_full[:, isl], dtype=np.float32),
            "ucorr": np.ascontiguousarray(ucorr_full[:, isl], dtype=np.float32),
            "qgt": _tile_gu(qg[isl], cfg),
            "qut": _tile_gu(qu[isl], cfg),
            "qdt": _tile_d(qd[:, isl], cfg),
            "sg": np.ascontiguousarray(
                np.broadcast_to(sgf[isl, 0], (cfg.B, cfg.I_SH))),
            "su": np.ascontiguousarray(
                np.broadcast_to(suf[isl, 0], (cfg.B, cfg.I_SH))),
        })
    return in_maps, sdf[:, 0]


_NC_CACHE = {}


def _get_nc(cfg):
    key = (cfg.B, cfg.H, cfg.I_FULL, cfg.WBLK, cfg.DBLK)
    if key not in _NC_CACHE:
        nc = bacc.Bacc(None, target_bir_lowering=False)
        build(nc, cfg)
        _NC_CACHE[key] = nc
    return _NC_CACHE[key]


def run(x, w_gate, w_up, w_down, cfg=FULL, **spmd_kwargs):
    """Full pipeline; returns (output [B,1,H] f32, BassKernelResults)."""
    in_maps, sd = make_in_maps(x, w_gate, w_up, w_down, cfg)
    nc = _get_nc(cfg)
    res = bass_utils.run_bass_kernel_spmd(
        nc, in_maps, core_ids=list(range(NCORES)), **spmd_kwargs)
    acc = np.zeros((cfg.B, cfg.H), np.float32)
    for r in res.results:
        acc += r["y"]
    out = (acc * sd[None, :]).astype(np.float32)
    return out.reshape(cfg.B, 1, cfg.H), res


def kernel(x, w_gate, w_up, w_down):
    out, _ = run(x, w_gate, w_up, w_down)
    return out



# revision 2
# speedup vs baseline: 2.1888x; 2.1888x over previous
"""Trainium2 Bass kernel for nn_LlamaQuantizedMLP (int4 fake-quant SwiGLU MLP).

Strategy (v2: fp8 stationary weights + FWL)
-------------------------------------------
Reference: per-row int4 fake quant of each weight (scale = max|w|/7,
q = clip(round(w/scale), -8, 7), w' = q*scale), then
  gate = x @ wg'.T ; up = x @ wu'.T ; h = silu(gate)*up ; y = h @ wd'.T

Int4 q values lie in [-8, 7] and are *exactly* representable in fp8 e4m3,
so the weights ship to HBM as fp8 — half the bytes of bf16, and the HBM
stream is the roofline for this problem (~17 MB/core @ ~358 GB/s).

On the PE, the fp8 weight tile is the *stationary* operand (128x128,
Fast-Weight-Load ingests fp8 at 4 elem/cycle/partition = 2x the rate of
streaming bf16 as the moving operand), while the tiny activations
(x / h, 8 columns) are the moving operand.  Per-row weight scales are
applied after the matmuls (gate/up: on device before SwiGLU; down: on
host after the cross-core reduction).  x and h are bf16 (mixed
fp8-stationary x bf16-moving matmul accumulates in fp32).

This orientation produces gate/up/h directly in [i-partition, batch]
layout — exactly the rhs layout the down matmul needs — so no on-device
transpose at all.

Sharding: tensor parallel over the intermediate dim (11008 = 8 x 1376,
zero-padded to 11 tiles of 128 per core).  Each core emits a partial
[4096, 8] output; the host sums the 8 partials and applies down scales.

All weights live in one [128, 1056, 128] fp8 tensor ordered exactly in
PE consumption order; DMA streams it in large blocks, double-buffered.
PSUM accumulation groups are strictly sequential and each owns a full
2KB bank (matmul start=True clears has_written for the whole bank).
"""

import numpy as np
import ml_dtypes

import concourse.bacc as bacc
import concourse.mybir as mybir
from concourse.tile import TileContext
from concourse import bass_utils

BF16 = mybir.dt.bfloat16
F32 = mybir.dt.float32
FP8 = mybir.dt.float8e4
NP_BF16 = ml_dtypes.bfloat16
NP_FP8 = ml_dtypes.float8_e4m3

NCORES = 8


class Cfg:
    def __init__(self, b=8, h=4096, i_full=11008, blk=88, wbufs=4):
        assert h % 128 == 0 and i_full % NCORES == 0
        self.B = b                      # batch = moving-operand columns
        self.H = h
        self.I_FULL = i_full
        self.I_SH = i_full // NCORES    # 1376 per core
        self.IT = (self.I_SH + 127) // 128   # 11 i-tiles (padded)
        self.I_PAD = self.IT * 128      # 1408
        self.KC = h // 128              # 32 contraction chunks (gate/up)
        self.HT = h // 128              # 32 output tiles (down)
        self.NGU = self.IT * 2 * self.KC     # 704 gate/up weight tiles
        self.ND = self.IT * self.HT          # 352 down weight tiles
        self.NW = self.NGU + self.ND         # 1056 total
        self.BLK = blk                  # weight tiles per DMA block
        self.NBLK = (self.NW + blk - 1) // blk
        self.WBUFS = wbufs


FULL = Cfg()


def build(nc, cfg):
    """Per-core SPMD program (identical on all cores; data differs)."""
    B, IT, KC, HT = cfg.B, cfg.IT, cfg.KC, cfg.HT

    w_all = nc.dram_tensor("w_all", [128, cfg.NW, 128], FP8,
                           kind="ExternalInput")
    xt = nc.dram_tensor("xt", [128, KC, B], BF16, kind="ExternalInput")
    sgb = nc.dram_tensor("sgb", [128, IT * B], F32, kind="ExternalInput")
    sub = nc.dram_tensor("sub", [128, IT * B], F32, kind="ExternalInput")
    y2 = nc.dram_tensor("y2", [128, HT * B], F32, kind="ExternalOutput")

    with TileContext(nc) as tc:
        with (
            tc.tile_pool(name="xs", bufs=1) as xs_pool,
            tc.tile_pool(name="w", bufs=cfg.WBUFS) as w_pool,
            tc.tile_pool(name="act", bufs=1) as act_pool,
            tc.tile_pool(name="psg", bufs=2, space="PSUM") as psg_pool,
            tc.tile_pool(name="psu", bufs=2, space="PSUM") as psu_pool,
            tc.tile_pool(name="psy", bufs=2, space="PSUM") as psy_pool,
        ):
            x_t = xs_pool.tile([128, KC, B], BF16, tag="x")
            nc.sync.dma_start(out=x_t[:], in_=xt[:])
            sg_t = xs_pool.tile([128, IT * B], F32, tag="sg")
            nc.sync.dma_start(out=sg_t[:], in_=sgb[:])
            su_t = xs_pool.tile([128, IT * B], F32, tag="su")
            nc.sync.dma_start(out=su_t[:], in_=sub[:])

            # weight block streaming state
            blocks = [None] * cfg.NBLK

            def wtile(seq):
                bi, off = divmod(seq, cfg.BLK)
                if blocks[bi] is None:
                    nt = min(cfg.BLK, cfg.NW - bi * cfg.BLK)
                    wb = w_pool.tile([128, cfg.BLK, 128], FP8, tag="wb")
                    nc.sync.dma_start(
                        out=wb[:, 0:nt, :],
                        in_=w_all[:, bi * cfg.BLK:bi * cfg.BLK + nt, :])
                    blocks[bi] = wb
                return blocks[bi][:, off, :]

            g_sb = act_pool.tile([128, IT * B], F32, tag="gsb")
            u_sb = act_pool.tile([128, IT * B], F32, tag="usb")

            # ---------------- phase 1: gate & up ----------------
            for it in range(IT):
                ps_g = psg_pool.tile([128, 512], F32, tag="g")
                ps_u = psu_pool.tile([128, 512], F32, tag="u")
                for gu, ps in ((0, ps_g), (1, ps_u)):
                    for k in range(KC):
                        seq = (it * 2 + gu) * KC + k
                        nc.tensor.matmul(
                            ps[:, 0:B], wtile(seq), x_t[:, k, :],
                            start=(k == 0), stop=(k == KC - 1))
                # stage to SBUF with the per-row quant scale fused in
                nc.vector.tensor_mul(
                    out=g_sb[:, it * B:(it + 1) * B], in0=ps_g[:, 0:B],
                    in1=sg_t[:, it * B:(it + 1) * B])
                nc.vector.tensor_mul(
                    out=u_sb[:, it * B:(it + 1) * B], in0=ps_u[:, 0:B],
                    in1=su_t[:, it * B:(it + 1) * B])

            # ---------------- SwiGLU (batched over all i-tiles) --------
            sig = act_pool.tile([128, IT * B], F32, tag="sig")
            nc.scalar.activation(
                out=sig[:], in_=g_sb[:],
                func=mybir.ActivationFunctionType.Sigmoid)
            silu = act_pool.tile([128, IT * B], F32, tag="silu")
            nc.vector.tensor_mul(out=silu[:], in0=g_sb[:], in1=sig[:])
            h_bf = act_pool.tile([128, IT * B], BF16, tag="hbf")
            nc.vector.tensor_mul(out=h_bf[:], in0=silu[:], in1=u_sb[:])

            # ---------------- phase 2: down ----------------
            y_sb = act_pool.tile([128, HT * B], F32, tag="ysb")
            for ht in range(HT):
                ps_y = psy_pool.tile([128, 512], F32, tag="y")
                for ik in range(IT):
                    seq = cfg.NGU + ht * IT + ik
                    nc.tensor.matmul(
                        ps_y[:, 0:B], wtile(seq),
                        h_bf[:, ik * B:(ik + 1) * B],
                        start=(ik == 0), stop=(ik == IT - 1))
                nc.vector.tensor_copy(
                    out=y_sb[:, ht * B:(ht + 1) * B], in_=ps_y[:, 0:B])
            nc.sync.dma_start(out=y2[:], in_=y_sb[:])

    nc.compile()
    return nc


# ---------------------------------------------------------------------------
# host-side preparation
# ---------------------------------------------------------------------------

def _quant(w):
    """Reference int4 fake-quant: integer q (f32) and per-row scale."""
    w = np.asarray(w, np.float32)
    scale = (np.max(np.abs(w), axis=1, keepdims=True) /
             np.float32(7.0)).astype(np.float32)
    scale = np.maximum(scale, np.float32(np.finfo(np.float32).tiny))
    q = np.clip(np.round((w / scale).astype(np.float32)), -8.0, 7.0).astype(
        np.float32)
    return q, scale


def make_in_maps(x, w_gate, w_up, w_down, cfg):
    """Returns (in_maps for 8 cores, down-scale vector [H])."""
    B, H, IT, KC, HT = cfg.B, cfg.H, cfg.IT, cfg.KC, cfg.HT
    qg, sgf = _quant(w_gate)
    qu, suf = _quant(w_up)
    qd, sdf = _quant(w_down)

    # x: [B,1,H] f32 -> [128, KC, B] bf16  ([r,k,b] = x[b, k*128+r])
    x2 = np.asarray(x, np.float32).reshape(B, H)
    xt = np.ascontiguousarray(
        x2.T.reshape(KC, 128, B).transpose(1, 0, 2).astype(NP_BF16))

    in_maps = []
    for c in range(NCORES):
        isl = slice(c * cfg.I_SH, (c + 1) * cfg.I_SH)

        def pad_i_rows(q_sh):          # [I_SH, H] -> [I_PAD, H]
            out = np.zeros((cfg.I_PAD, H), np.float32)
            out[0:cfg.I_SH] = q_sh
            return out

        qg_sh = pad_i_rows(qg[isl])
        qu_sh = pad_i_rows(qu[isl])
        qd_sh = np.zeros((H, cfg.I_PAD), np.float32)
        qd_sh[:, 0:cfg.I_SH] = qd[:, isl]

        # gate/up tiles: [r, it, gu, k, c] = q[it*128+c, k*128+r]
        def gu_tiles(q_sh):            # [I_PAD, H] -> [128, IT, KC, 128]
            return q_sh.reshape(IT, 128, KC, 128).transpose(3, 0, 2, 1)

        wgu = np.stack([gu_tiles(qg_sh), gu_tiles(qu_sh)], axis=2)
        wgu = wgu.reshape(128, cfg.NGU, 128)
        # down tiles: [r, ht, ik, c] = qd[ht*128+c, ik*128+r]
        wd = qd_sh.reshape(HT, 128, IT, 128).transpose(3, 0, 2, 1)
        wd = wd.reshape(128, cfg.ND, 128)
        w_all = np.ascontiguousarray(
            np.concatenate([wgu, wd], axis=1).astype(NP_FP8))

        # per-row scales, broadcast over batch: [r, it*B+b] = s[it*128+r]
        def sc_b(s_col):               # [I_SH,1] -> [128, IT*B] f32
            s_pad = np.zeros((cfg.I_PAD,), np.float32)
            s_pad[0:cfg.I_SH] = s_col[:, 0]
            return np.ascontiguousarray(np.broadcast_to(
                s_pad.reshape(IT, 128, 1).transpose(1, 0, 2),
                (128, IT, B)).reshape(128, IT * B))

        in_maps.append({
            "w_all": w_all,
            "xt": xt,
            "sgb": sc_b(sgf[isl]),
            "sub": sc_b(suf[isl]),
        })
    return in_maps, sdf[:, 0]


_NC_CACHE = {}


def _get_nc(cfg):
    key = (cfg.B, cfg.H, cfg.I_FULL, cfg.BLK, cfg.WBUFS)
    if key not in _NC_CACHE:
        nc = bacc.Bacc(None, target_bir_lowering=False)
        build(nc, cfg)
        _NC_CACHE[key] = nc
    return _NC_CACHE[key]


def run(x, w_gate, w_up, w_down, cfg=FULL, **spmd_kwargs):
    """Full pipeline; returns (output [B,1,H] f32, BassKernelResults)."""
    in_maps, sd = make_in_maps(x, w_gate, w_up, w_down, cfg)
    nc = _get_nc(cfg)
    res = bass_utils.run_bass_kernel_spmd(
        nc, in_maps, core_ids=list(range(NCORES)), **spmd_kwargs)
    acc = np.zeros((128, cfg.HT * cfg.B), np.float32)
    for r in res.results:
        acc += r["y2"]
    # y2 [r, ht*B+b] = partial y[b, ht*128+r]
    y = acc.reshape(128, cfg.HT, cfg.B).transpose(2, 1, 0).reshape(
        cfg.B, cfg.H)
    y = y * sd[None, :]
    return y.reshape(cfg.B, 1, cfg.H).astype(np.float32), res


def kernel(x, w_gate, w_up, w_down):
    out, _ = run(x, w_gate, w_up, w_down)
    return out


# revision 4
# speedup vs baseline: 2.2491x; 1.0275x over previous
"""Trainium2 Bass kernel for nn_LlamaQuantizedMLP (int4 fake-quant SwiGLU MLP).

Strategy (v3: fp8 stationary weights + FWL, packed PSUM banks)
--------------------------------------------------------------
Reference: per-row int4 fake quant of each weight (scale = max|w|/7,
q = clip(round(w/scale), -8, 7), w' = q*scale), then
  gate = x @ wg'.T ; up = x @ wu'.T ; h = silu(gate)*up ; y = h @ wd'.T

Int4 q values lie in [-8, 7] and are *exactly* representable in fp8 e4m3,
so the weights ship to HBM as fp8 — half the bytes of bf16, and the HBM
stream is the roofline for this problem (~17 MB/core @ ~390 GB/s).

On the PE, the fp8 weight tile is the *stationary* operand (128x128;
Fast-Weight-Load ingests fp8 at 4 elem/cycle/partition = 2x the rate of
streaming bf16 as the moving operand), while the tiny activations (x/h,
8 bf16 columns) are the moving operand.  Per-row weight scales are
applied after the matmuls (gate/up: on device before SwiGLU; down: on
host after the cross-core reduction).  This orientation produces
gate/up/h directly in [i-partition, batch] layout — exactly the rhs
layout the down matmul needs — no on-device transpose at all.

Pipelining details:
 - All weights live in one [128, 1056, 128] fp8 tensor ordered exactly
   in PE consumption order, streamed in blocks on the Sync HWDGE queue
   (issued before anything else so the first weight byte moves ASAP);
   x / scales / outputs ride the Scalar HWDGE queue.  First + last
   blocks are small so the PE starts early and drains fast.
 - Matmul accumulation groups are strictly sequential, so many groups
   share one PSUM bank at different column offsets (start=True clears
   only the has_written *bits* of the bank; completed groups' values
   survive).  DVE then reads whole banks in a handful of wide ops,
   never while the PE is writing that bank (fatal collision otherwise).

Sharding: tensor parallel over the intermediate dim (11008 = 8 x 1376,
zero-padded to 11 tiles of 128 per core).  Each core emits a partial
[4096, 8] output; the host sums the 8 partials and applies down scales.
"""

import numpy as np
import ml_dtypes

import concourse.bacc as bacc
import concourse.mybir as mybir
from concourse.tile import TileContext
from concourse import bass_utils

BF16 = mybir.dt.bfloat16
F32 = mybir.dt.float32
FP8 = mybir.dt.float8e4
NP_BF16 = ml_dtypes.bfloat16
NP_FP8 = ml_dtypes.float8_e4m3

NCORES = 8


def _block_plan(nw):
    """DMA block sizes (in 16KB weight tiles) summing to nw: small head
    so the PE starts early, small tail so the PE drains fast."""
    if nw <= 100:
        return [nw]
    plan = [16, 72]
    rem = nw - 88
    while rem > 176:
        plan.append(88)
        rem -= 88
    for s in (72, 56, 32, 16, 8, 4, 2, 1):
        while rem >= s:
            plan.append(s)
            rem -= s
    assert sum(plan) == nw
    return plan


class Cfg:
    def __init__(self, b=8, h=4096, i_full=11008, wbufs=4):
        assert h % 128 == 0 and i_full % NCORES == 0
        self.B = b                      # batch = moving-operand columns
        self.H = h
        self.I_FULL = i_full
        self.I_SH = i_full // NCORES    # 1376 per core
        self.IT = (self.I_SH + 127) // 128   # 11 i-tiles (padded)
        self.I_PAD = self.IT * 128      # 1408
        self.KC = h // 128              # 32 contraction chunks (gate/up)
        self.HT = h // 128              # 32 output tiles (down)
        self.NGU = self.IT * 2 * self.KC     # 704 gate/up weight tiles
        self.ND = self.IT * self.HT          # 352 down weight tiles
        self.NW = self.NGU + self.ND         # 1056 total
        self.PLAN = _block_plan(self.NW)
        self.BLKMAX = max(self.PLAN)
        self.WBUFS = min(wbufs, len(self.PLAN))
        # i-tile halves for overlapped SwiGLU staging
        self.IT_A = (self.IT + 1) // 2
        # down output chunks (PSUM banks): groups of ht tiles
        self.YCH = 2 if self.HT % 2 == 0 else 1
        self.HT_C = self.HT // self.YCH
        assert self.HT_C * self.B <= 512


FULL = Cfg()


def build(nc, cfg):
    """Per-core SPMD program (identical on all cores; data differs)."""
    B, IT, KC, HT = cfg.B, cfg.IT, cfg.KC, cfg.HT

    w_all = nc.dram_tensor("w_all", [128, cfg.NW, 128], FP8,
                           kind="ExternalInput")
    xt = nc.dram_tensor("xt", [128, KC, B], BF16, kind="ExternalInput")
    sgb = nc.dram_tensor("sgb", [128, IT * B], F32, kind="ExternalInput")
    sub = nc.dram_tensor("sub", [128, IT * B], F32, kind="ExternalInput")
    y2 = nc.dram_tensor("y2", [128, HT * B], F32, kind="ExternalOutput")

    with TileContext(nc) as tc:
        with (
            tc.tile_pool(name="xs", bufs=1) as xs_pool,
            tc.tile_pool(name="w", bufs=cfg.WBUFS) as w_pool,
            tc.tile_pool(name="act", bufs=1) as act_pool,
            tc.tile_pool(name="ps", bufs=1, space="PSUM") as ps_pool,
        ):
            # ---- weight stream first: sync HWDGE queue is FIFO and the
            # first weight block gates the first matmul.
            blocks = []
            b0 = 0
            for nt in cfg.PLAN:
                wb = w_pool.tile([128, cfg.BLKMAX, 128], FP8, tag="wb")
                nc.sync.dma_start(out=wb[:, 0:nt, :],
                                  in_=w_all[:, b0:b0 + nt, :])
                blocks.append((b0, nt, wb))
                b0 += nt

            def wtile(seq):
                for b0, nt, wb in blocks:
                    if seq < b0 + nt:
                        return wb[:, seq - b0, :]
                raise AssertionError(seq)

            # ---- small inputs on the scalar HWDGE queue
            x_t = xs_pool.tile([128, KC, B], BF16, tag="x")
            nc.scalar.dma_start(out=x_t[:], in_=xt[:])
            sg_t = xs_pool.tile([128, IT * B], F32, tag="sg")
            nc.scalar.dma_start(out=sg_t[:], in_=sgb[:])
            su_t = xs_pool.tile([128, IT * B], F32, tag="su")
            nc.scalar.dma_start(out=su_t[:], in_=sub[:])

            h_bf = act_pool.tile([128, IT * B], BF16, tag="hbf")

            # ---------------- phase 1: gate & up ----------------
            # i-tile halves; each half packs its gate (up) groups into
            # one PSUM bank, SwiGLU of half A overlaps PE of half B.
            for ha, (i0, i1) in enumerate(
                    ((0, cfg.IT_A), (cfg.IT_A, IT))):
                nit = i1 - i0
                if nit == 0:
                    continue
                ps_g = ps_pool.tile([128, 512], F32, tag=f"g{ha}")
                ps_u = ps_pool.tile([128, 512], F32, tag=f"u{ha}")
                for it in range(i0, i1):
                    for gu, ps in ((0, ps_g), (1, ps_u)):
                        c0 = (it - i0) * B
                        for k in range(KC):
                            seq = (it * 2 + gu) * KC + k
                            nc.tensor.matmul(
                                ps[:, c0:c0 + B], wtile(seq), x_t[:, k, :],
                                start=(k == 0), stop=(k == KC - 1))
                # ---- SwiGLU for this half (reads full banks once; DVE
                # touches at most one PSUM operand per instruction)
                sl = slice(i0 * B, i1 * B)
                g_sb = act_pool.tile([128, IT * B], F32, tag="gsb")
                nc.vector.tensor_mul(out=g_sb[:, 0:nit * B],
                                     in0=ps_g[:, 0:nit * B], in1=sg_t[:, sl])
                u_sb = act_pool.tile([128, IT * B], F32, tag="usb")
                nc.vector.tensor_mul(out=u_sb[:, 0:nit * B],
                                     in0=ps_u[:, 0:nit * B], in1=su_t[:, sl])
                sig = act_pool.tile([128, IT * B], F32, tag="sig")
                nc.scalar.activation(
                    out=sig[:, 0:nit * B], in_=g_sb[:, 0:nit * B],
                    func=mybir.ActivationFunctionType.Sigmoid)
                silu = act_pool.tile([128, IT * B], F32, tag="silu")
                nc.vector.tensor_mul(out=silu[:, 0:nit * B],
                                     in0=g_sb[:, 0:nit * B],
                                     in1=sig[:, 0:nit * B])
                nc.vector.tensor_mul(out=h_bf[:, sl],
                                     in0=silu[:, 0:nit * B],
                                     in1=u_sb[:, 0:nit * B])

            # ---------------- phase 2: down ----------------
            y_sb = act_pool.tile([128, HT * B], F32, tag="ysb")
            for ch in range(cfg.YCH):
                ps_y = ps_pool.tile([128, 512], F32, tag=f"y{ch % 2}")
                for g in range(cfg.HT_C):
                    ht = ch * cfg.HT_C + g
                    for ik in range(IT):
                        seq = cfg.NGU + ht * IT + ik
                        nc.tensor.matmul(
                            ps_y[:, g * B:(g + 1) * B], wtile(seq),
                            h_bf[:, ik * B:(ik + 1) * B],
                            start=(ik == 0), stop=(ik == IT - 1))
                csl = slice(ch * cfg.HT_C * B, (ch + 1) * cfg.HT_C * B)
                nc.vector.tensor_copy(out=y_sb[:, csl],
                                      in_=ps_y[:, 0:cfg.HT_C * B])
                nc.scalar.dma_start(out=y2[:, csl], in_=y_sb[:, csl])

    nc.compile()
    return nc


# ---------------------------------------------------------------------------
# host-side preparation
# ---------------------------------------------------------------------------

def _quant(w):
    """Reference int4 fake-quant: integer q (f32) and per-row scale."""
    w = np.asarray(w, np.float32)
    scale = (np.max(np.abs(w), axis=1, keepdims=True) /
             np.float32(7.0)).astype(np.float32)
    scale = np.maximum(scale, np.float32(np.finfo(np.float32).tiny))
    q = np.clip(np.round((w / scale).astype(np.float32)), -8.0, 7.0).astype(
        np.float32)
    return q, scale


def make_in_maps(x, w_gate, w_up, w_down, cfg):
    """Returns (in_maps for 8 cores, down-scale vector [H])."""
    B, H, IT, KC, HT = cfg.B, cfg.H, cfg.IT, cfg.KC, cfg.HT
    qg, sgf = _quant(w_gate)
    qu, suf = _quant(w_up)
    qd, sdf = _quant(w_down)

    # x: [B,1,H] f32 -> [128, KC, B] bf16  ([r,k,b] = x[b, k*128+r])
    x2 = np.asarray(x, np.float32).reshape(B, H)
    xt = np.ascontiguousarray(
        x2.T.reshape(KC, 128, B).transpose(1, 0, 2).astype(NP_BF16))

    in_maps = []
    for c in range(NCORES):
        isl = slice(c * cfg.I_SH, (c + 1) * cfg.I_SH)

        def pad_i_rows(q_sh):          # [I_SH, H] -> [I_PAD, H]
            out = np.zeros((cfg.I_PAD, H), np.float32)
            out[0:cfg.I_SH] = q_sh
            return out

        qg_sh = pad_i_rows(qg[isl])
        qu_sh = pad_i_rows(qu[isl])
        qd_sh = np.zeros((H, cfg.I_PAD), np.float32)
        qd_sh[:, 0:cfg.I_SH] = qd[:, isl]

        # gate/up tiles: [r, it, gu, k, c] = q[it*128+c, k*128+r]
        def gu_tiles(q_sh):            # [I_PAD, H] -> [128, IT, KC, 128]
            return q_sh.reshape(IT, 128, KC, 128).transpose(3, 0, 2, 1)

        wgu = np.stack([gu_tiles(qg_sh), gu_tiles(qu_sh)], axis=2)
        wgu = wgu.reshape(128, cfg.NGU, 128)
        # down tiles: [r, ht, ik, c] = qd[ht*128+c, ik*128+r]
        wd = qd_sh.reshape(HT, 128, IT, 128).transpose(3, 0, 2, 1)
        wd = wd.reshape(128, cfg.ND, 128)
        w_all = np.ascontiguousarray(
            np.concatenate([wgu, wd], axis=1).astype(NP_FP8))

        # per-row scales, broadcast over batch: [r, it*B+b] = s[it*128+r]
        def sc_b(s_col):               # [I_SH,1] -> [128, IT*B] f32
            s_pad = np.zeros((cfg.I_PAD,), np.float32)
            s_pad[0:cfg.I_SH] = s_col[:, 0]
            return np.ascontiguousarray(np.broadcast_to(
                s_pad.reshape(IT, 128, 1).transpose(1, 0, 2),
                (128, IT, B)).reshape(128, IT * B))

        in_maps.append({
            "w_all": w_all,
            "xt": xt,
            "sgb": sc_b(sgf[isl]),
            "sub": sc_b(suf[isl]),
        })
    return in_maps, sdf[:, 0]


_NC_CACHE = {}


def _get_nc(cfg):
    key = (cfg.B, cfg.H, cfg.I_FULL, cfg.WBUFS)
    if key not in _NC_CACHE:
        nc = bacc.Bacc(None, target_bir_lowering=False)
        build(nc, cfg)
        _NC_CACHE[key] = nc
    return _NC_CACHE[key]


def run(x, w_gate, w_up, w_down, cfg=FULL, **spmd_kwargs):
    """Full pipeline; returns (output [B,1,H] f32, BassKernelResults)."""
    in_maps, sd = make_in_maps(x, w_gate, w_up, w_down, cfg)
    nc = _get_nc(cfg)
    res = bass_utils.run_bass_kernel_spmd(
        nc, in_maps, core_ids=list(range(NCORES)), **spmd_kwargs)
    acc = np.zeros((128, cfg.HT * cfg.B), np.float32)
    for r in res.results:
        acc += r["y2"]
    # y2 [r, ht*B+b] = partial y[b, ht*128+r]
    y = acc.reshape(128, cfg.HT, cfg.B).transpose(2, 1, 0).reshape(
        cfg.B, cfg.H)
    y = y * sd[None, :]
    return y.reshape(cfg.B, 1, cfg.H).astype(np.float32), res


def kernel(x, w_gate, w_up, w_down):
    out, _ = run(x, w_gate, w_up, w_down)
    return out


# revision 8
# speedup vs baseline: 2.2831x; 1.0151x over previous
"""Trainium2 Bass kernel for nn_LlamaQuantizedMLP (int4 fake-quant SwiGLU MLP).

Strategy (v3: fp8 stationary weights + FWL, packed PSUM banks)
--------------------------------------------------------------
Reference: per-row int4 fake quant of each weight (scale = max|w|/7,
q = clip(round(w/scale), -8, 7), w' = q*scale), then
  gate = x @ wg'.T ; up = x @ wu'.T ; h = silu(gate)*up ; y = h @ wd'.T

Int4 q values lie in [-8, 7] and are *exactly* representable in fp8 e4m3,
so the weights ship to HBM as fp8 — half the bytes of bf16, and the HBM
stream is the roofline for this problem (~17 MB/core @ ~390 GB/s).

On the PE, the fp8 weight tile is the *stationary* operand (128x128;
Fast-Weight-Load ingests fp8 at 4 elem/cycle/partition = 2x the rate of
streaming bf16 as the moving operand), while the tiny activations (x/h,
8 bf16 columns) are the moving operand.  Per-row weight scales are
applied after the matmuls (gate/up: on device before SwiGLU; down: on
host after the cross-core reduction).  This orientation produces
gate/up/h directly in [i-partition, batch] layout — exactly the rhs
layout the down matmul needs — no on-device transpose at all.

Pipelining details:
 - All weights live in one [128, 1056, 128] fp8 tensor ordered exactly
   in PE consumption order, streamed in blocks on the Sync HWDGE queue
   (issued before anything else so the first weight byte moves ASAP);
   x / scales / outputs ride the Scalar HWDGE queue.  First + last
   blocks are small so the PE starts early and drains fast.
 - Matmul accumulation groups are strictly sequential, so many groups
   share one PSUM bank at different column offsets (start=True clears
   only the has_written *bits* of the bank; completed groups' values
   survive).  DVE then reads whole banks in a handful of wide ops,
   never while the PE is writing that bank (fatal collision otherwise).

Sharding: tensor parallel over the intermediate dim (11008 = 8 x 1376,
zero-padded to 11 tiles of 128 per core).  Each core emits a partial
[4096, 8] output; the host sums the 8 partials and applies down scales.
"""

import numpy as np
import ml_dtypes

import concourse.bacc as bacc
import concourse.mybir as mybir
from concourse.tile import TileContext
from concourse import bass_utils

BF16 = mybir.dt.bfloat16
F32 = mybir.dt.float32
FP8 = mybir.dt.float8e4
NP_BF16 = ml_dtypes.bfloat16
NP_FP8 = ml_dtypes.float8_e4m3

NCORES = 8


def _block_plan(nw):
    """DMA block sizes (in 16KB weight tiles) summing to nw: small head
    so the PE starts early, small tail so the PE drains fast."""
    if nw <= 100:
        return [nw]
    plan = [16, 72]
    rem = nw - 88
    while rem > 176:
        plan.append(88)
        rem -= 88
    for s in (64, 48, 32, 16, 8, 4, 2, 1):
        while rem >= s:
            plan.append(s)
            rem -= s
    assert sum(plan) == nw
    return plan


class Cfg:
    def __init__(self, b=8, h=4096, i_full=11008, wbufs=4):
        assert h % 128 == 0 and i_full % NCORES == 0
        self.B = b                      # batch = moving-operand columns
        self.H = h
        self.I_FULL = i_full
        self.I_SH = i_full // NCORES    # 1376 per core
        self.IT = (self.I_SH + 127) // 128   # 11 i-tiles (padded)
        self.I_PAD = self.IT * 128      # 1408
        self.KC = h // 128              # 32 contraction chunks (gate/up)
        self.HT = h // 128              # 32 output tiles (down)
        self.NGU = self.IT * 2 * self.KC     # 704 gate/up weight tiles
        self.ND = self.IT * self.HT          # 352 down weight tiles
        self.NW = self.NGU + self.ND         # 1056 total
        self.PLAN = _block_plan(self.NW)
        self.BLKMAX = max(self.PLAN)
        self.WBUFS = min(wbufs, len(self.PLAN))
        # i-tile halves for overlapped SwiGLU staging
        self.IT_A = (self.IT + 1) // 2
        # down output chunks (PSUM banks): groups of ht tiles
        self.YCH = 4 if self.HT % 4 == 0 else 1
        self.HT_C = self.HT // self.YCH
        assert self.HT_C * self.B <= 512


FULL = Cfg()


def build(nc, cfg):
    """Per-core SPMD program (identical on all cores; data differs)."""
    B, IT, KC, HT = cfg.B, cfg.IT, cfg.KC, cfg.HT

    w_all = nc.dram_tensor("w_all", [128, cfg.NW, 128], FP8,
                           kind="ExternalInput")
    xt = nc.dram_tensor("xt", [128, KC, B], BF16, kind="ExternalInput")
    sgb = nc.dram_tensor("sgb", [128, IT * B], F32, kind="ExternalInput")
    sub = nc.dram_tensor("sub", [128, IT * B], F32, kind="ExternalInput")
    y2 = nc.dram_tensor("y2", [128, HT * B], F32, kind="ExternalOutput")

    with TileContext(nc) as tc:
        with (
            tc.tile_pool(name="xs", bufs=1) as xs_pool,
            tc.tile_pool(name="w", bufs=1) as w_pool,
            tc.tile_pool(name="act", bufs=1) as act_pool,
            tc.tile_pool(name="ps", bufs=1, space="PSUM") as ps_pool,
        ):
            # ---- weight stream first: sync HWDGE queue is FIFO and the
            # first weight block gates the first matmul.  Every block has
            # its own SBUF slot (all weights fit), so the DMA stream is
            # never gated on PE progress.
            blocks = []
            b0 = 0
            for bi, nt in enumerate(cfg.PLAN):
                wb = w_pool.tile([128, nt, 128], FP8, tag=f"wb{bi}")
                nc.sync.dma_start(out=wb[:],
                                  in_=w_all[:, b0:b0 + nt, :])
                blocks.append((b0, nt, wb))
                b0 += nt

            def wtile(seq):
                for b0, nt, wb in blocks:
                    if seq < b0 + nt:
                        return wb[:, seq - b0, :]
                raise AssertionError(seq)

            # ---- small inputs on the scalar HWDGE queue
            x_t = xs_pool.tile([128, KC, B], BF16, tag="x")
            nc.scalar.dma_start(out=x_t[:], in_=xt[:])
            sg_t = xs_pool.tile([128, IT * B], F32, tag="sg")
            nc.scalar.dma_start(out=sg_t[:], in_=sgb[:])
            su_t = xs_pool.tile([128, IT * B], F32, tag="su")
            nc.scalar.dma_start(out=su_t[:], in_=sub[:])

            h_bf = act_pool.tile([128, IT * B], BF16, tag="hbf")

            # ---------------- phase 1: gate & up ----------------
            # i-tile halves; each half packs its gate (up) groups into
            # one PSUM bank, SwiGLU of half A overlaps PE of half B.
            for ha, (i0, i1) in enumerate(
                    ((0, cfg.IT_A), (cfg.IT_A, IT))):
                nit = i1 - i0
                if nit == 0:
                    continue
                ps_g = ps_pool.tile([128, 512], F32, tag=f"g{ha}")
                ps_u = ps_pool.tile([128, 512], F32, tag=f"u{ha}")
                for it in range(i0, i1):
                    for gu, ps in ((0, ps_g), (1, ps_u)):
                        c0 = (it - i0) * B
                        for k in range(KC):
                            seq = (it * 2 + gu) * KC + k
                            nc.tensor.matmul(
                                ps[:, c0:c0 + B], wtile(seq), x_t[:, k, :],
                                start=(k == 0), stop=(k == KC - 1))
                # ---- SwiGLU for this half (reads full banks once; DVE
                # touches at most one PSUM operand per instruction)
                sl = slice(i0 * B, i1 * B)
                g_sb = act_pool.tile([128, IT * B], F32, tag="gsb")
                nc.vector.tensor_mul(out=g_sb[:, 0:nit * B],
                                     in0=ps_g[:, 0:nit * B], in1=sg_t[:, sl])
                u_sb = act_pool.tile([128, IT * B], F32, tag="usb")
                nc.vector.tensor_mul(out=u_sb[:, 0:nit * B],
                                     in0=ps_u[:, 0:nit * B], in1=su_t[:, sl])
                sig = act_pool.tile([128, IT * B], F32, tag="sig")
                nc.scalar.activation(
                    out=sig[:, 0:nit * B], in_=g_sb[:, 0:nit * B],
                    func=mybir.ActivationFunctionType.Sigmoid)
                silu = act_pool.tile([128, IT * B], F32, tag="silu")
                nc.vector.tensor_mul(out=silu[:, 0:nit * B],
                                     in0=g_sb[:, 0:nit * B],
                                     in1=sig[:, 0:nit * B])
                nc.vector.tensor_mul(out=h_bf[:, sl],
                                     in0=silu[:, 0:nit * B],
                                     in1=u_sb[:, 0:nit * B])

            # ---------------- phase 2: down ----------------
            y_sb = act_pool.tile([128, HT * B], F32, tag="ysb")
            for ch in range(cfg.YCH):
                ps_y = ps_pool.tile([128, 512], F32, tag=f"y{ch % 2}")
                for g in range(cfg.HT_C):
                    ht = ch * cfg.HT_C + g
                    for ik in range(IT):
                        seq = cfg.NGU + ht * IT + ik
                        nc.tensor.matmul(
                            ps_y[:, g * B:(g + 1) * B], wtile(seq),
                            h_bf[:, ik * B:(ik + 1) * B],
                            start=(ik == 0), stop=(ik == IT - 1))
                csl = slice(ch * cfg.HT_C * B, (ch + 1) * cfg.HT_C * B)
                nc.vector.tensor_copy(out=y_sb[:, csl],
                                      in_=ps_y[:, 0:cfg.HT_C * B])
                nc.scalar.dma_start(out=y2[:, csl], in_=y_sb[:, csl])

    nc.compile()
    return nc


# ---------------------------------------------------------------------------
# host-side preparation
# ---------------------------------------------------------------------------

def _quant(w):
    """Reference int4 fake-quant: integer q (f32) and per-row scale."""
    w = np.asarray(w, np.float32)
    scale = (np.max(np.abs(w), axis=1, keepdims=True) /
             np.float32(7.0)).astype(np.float32)
    scale = np.maximum(scale, np.float32(np.finfo(np.float32).tiny))
    q = np.clip(np.round((w / scale).astype(np.float32)), -8.0, 7.0).astype(
        np.float32)
    return q, scale


def make_in_maps(x, w_gate, w_up, w_down, cfg):
    """Returns (in_maps for 8 cores, down-scale vector [H])."""
    B, H, IT, KC, HT = cfg.B, cfg.H, cfg.IT, cfg.KC, cfg.HT
    qg, sgf = _quant(w_gate)
    qu, suf = _quant(w_up)
    qd, sdf = _quant(w_down)

    # x: [B,1,H] f32 -> [128, KC, B] bf16  ([r,k,b] = x[b, k*128+r])
    x2 = np.asarray(x, np.float32).reshape(B, H)
    xt = np.ascontiguousarray(
        x2.T.reshape(KC, 128, B).transpose(1, 0, 2).astype(NP_BF16))

    in_maps = []
    for c in range(NCORES):
        isl = slice(c * cfg.I_SH, (c + 1) * cfg.I_SH)

        def pad_i_rows(q_sh):          # [I_SH, H] -> [I_PAD, H]
            out = np.zeros((cfg.I_PAD, H), np.float32)
            out[0:cfg.I_SH] = q_sh
            return out

        qg_sh = pad_i_rows(qg[isl])
        qu_sh = pad_i_rows(qu[isl])
        qd_sh = np.zeros((H, cfg.I_PAD), np.float32)
        qd_sh[:, 0:cfg.I_SH] = qd[:, isl]

        # gate/up tiles: [r, it, gu, k, c] = q[it*128+c, k*128+r]
        def gu_tiles(q_sh):            # [I_PAD, H] -> [128, IT, KC, 128]
            return q_sh.reshape(IT, 128, KC, 128).transpose(3, 0, 2, 1)

        wgu = np.stack([gu_tiles(qg_sh), gu_tiles(qu_sh)], axis=2)
        wgu = wgu.reshape(128, cfg.NGU, 128)
        # down tiles: [r, ht, ik, c] = qd[ht*128+c, ik*128+r]
        wd = qd_sh.reshape(HT, 128, IT, 128).transpose(3, 0, 2, 1)
        wd = wd.reshape(128, cfg.ND, 128)
        w_all = np.ascontiguousarray(
            np.concatenate([wgu, wd], axis=1).astype(NP_FP8))

        # per-row scales, broadcast over batch: [r, it*B+b] = s[it*128+r]
        def sc_b(s_col):               # [I_SH,1] -> [128, IT*B] f32
            s_pad = np.zeros((cfg.I_PAD,), np.float32)
            s_pad[0:cfg.I_SH] = s_col[:, 0]
            return np.ascontiguousarray(np.broadcast_to(
                s_pad.reshape(IT, 128, 1).transpose(1, 0, 2),
                (128, IT, B)).reshape(128, IT * B))

        in_maps.append({
            "w_all": w_all,
            "xt": xt,
            "sgb": sc_b(sgf[isl]),
            "sub": sc_b(suf[isl]),
        })
    return in_maps, sdf[:, 0]


_NC_CACHE = {}


def _get_nc(cfg):
    key = (cfg.B, cfg.H, cfg.I_FULL, cfg.WBUFS)
    if key not in _NC_CACHE:
        nc = bacc.Bacc(None, target_bir_lowering=False)
        build(nc, cfg)
        _NC_CACHE[key] = nc
    return _NC_CACHE[key]


def run(x, w_gate, w_up, w_down, cfg=FULL, **spmd_kwargs):
    """Full pipeline; returns (output [B,1,H] f32, BassKernelResults)."""
    in_maps, sd = make_in_maps(x, w_gate, w_up, w_down, cfg)
    nc = _get_nc(cfg)
    res = bass_utils.run_bass_kernel_spmd(
        nc, in_maps, core_ids=list(range(NCORES)), **spmd_kwargs)
    acc = np.zeros((128, cfg.HT * cfg.B), np.float32)
    for r in res.results:
        acc += r["y2"]
    # y2 [r, ht*B+b] = partial y[b, ht*128+r]
    y = acc.reshape(128, cfg.HT, cfg.B).transpose(2, 1, 0).reshape(
        cfg.B, cfg.H)
    y = y * sd[None, :]
    return y.reshape(cfg.B, 1, cfg.H).astype(np.float32), res


def kernel(x, w_gate, w_up, w_down):
    out, _ = run(x, w_gate, w_up, w_down)
    return out
